# revision 23
# baseline (speedup 1.0000x reference)
"""Multi-head softmax attention (b=4, s=2048, d=1024, 16 heads) on 8 trn2 cores.

Sharding: 2D over (batch, head-half). Core c handles batch c//2, heads
[8*(c%2), 8*(c%2)+8). Each core computes its QKV projections, attention for
its 8 heads, and a partial output projection (row-parallel over its 512
attn-out columns). Host sums the two partials per batch.

Device schedule: a single software-pipelined loop over (pair, qt, k-chunk)
groups. Scores for two consecutive groups are emitted back-to-back (one
64-row quadrant-config transition per two groups -- each config switch
costs ~95ns of un-prefetched LDWEIGHTS), their exps follow on the ACT
engine, and attn@V consumption trails by 4 groups so slower producers never
head-of-line-block the in-order PE queue. All projection work (V tiles,
next pair's Q^T/K^T, out-proj) is dripped between groups from a
deadline-ordered queue with forced drains before each consumer; evacuation
ops are never dripped in the same group as their matmul chain (they would
block the strict DVE FIFO while waiting). Input DMA uses coarse transfers
(dma_start costs ~600ns issue time each) split across both HW-DGE rings,
and a dummy matmul chain keeps the PE HAM-warm through the DMA wait. 16 of
the 256 exp chunks (pairs 1-2, where ACT would otherwise bind) run on the
DVE via a custom 2-instruction op computing exp(x) = (c*(x+h)^2+k)^256
(minimax quadratic + 8 squarings, rel err ~7e-4). Outputs are written f16
and summed on host.

Data path is fp16 (weights/activations/P), accumulation fp32 in PSUM,
softmax normalization in fp32. Softmax max-subtraction is skipped (scores
are O(5), exp is safe well within fp16/fp32 range).
"""

import numpy as np

HIDDEN = 1024
SEQ = 2048
BATCH = 4
HEADS = 16
HG = 8  # heads per core
HD = 64  # head dim

_CACHE = {}
_TRACE = False  # test.py sets this for profiling runs
LAST_RESULT = None

# exp(x) ~= (EC*(x+EH)^2 + EK)^256 on [-6.8, 6.8], rel err <= 7.1e-4
EC = 7.5799348451255355e-06
EH = 257.6974182128906
EK = 0.49663403630256653

# (pair, g) chunks whose exp runs on DVE instead of ACT; positioned late in
# each qt so their DVE ops never queue behind the previous qt's norm chain.
DVE_GS = (10, 13)
DVE_PAIRS = (1, 2)
DVE_CHUNKS = {(p, g) for p in DVE_PAIRS for g in DVE_GS}


def _register_dve_ops():
    from concourse import dve_ops
    from concourse.dve_spec import Spec, Src0, C0, C1, C2, sq

    if any(o.name == "EXP256_P1" for o in dve_ops.OPS):
        return

    u = Src0 + C0
    p = sq(u) * C1 + C2
    for _ in range(4):
        p = sq(p)
    spec1 = Spec(
        body=p,
        reference=lambda in0, in1, s0, s1, imm2: ((in0 + s0) ** 2 * s1 + imm2)
        ** 16,
    )
    q = Src0
    for _ in range(4):
        q = sq(q)
    spec2 = Spec(body=q, reference=lambda in0, in1, s0, s1, imm2: in0**16)

    for nm, sp, sha in [
        ("EXP256_P1", spec1, "481c0b961f8e522b"),
        ("EXP256_P2", spec2, "6d6edb7498c4a68d"),
    ]:
        op = dve_ops.DveOp(nm, sp, subdim=False, uops_sha={"v3": sha})
        dve_ops.OPS.append(op)
        dve_ops._SUB_OPCODE_FOR_NAME[nm] = (
            dve_ops._CUSTOM_DVE_ROW_BASE + len(dve_ops.OPS) - 1
        )


def _build_nc():
    from collections import deque

    import concourse.mybir as mybir
    import concourse.tile as tile
    from concourse import bacc, dve_ops

    _register_dve_ops()
    exp_p1 = next(o for o in dve_ops.OPS if o.name == "EXP256_P1")
    exp_p2 = next(o for o in dve_ops.OPS if o.name == "EXP256_P2")

    f32 = mybir.dt.float32
    f16 = mybir.dt.float16
    Exp = mybir.ActivationFunctionType.Exp

    nc = bacc.Bacc("TRN2", target_bir_lowering=False, debug=False)
    xT = nc.dram_tensor("xT", [128, 8, SEQ], f16, kind="ExternalInput").ap()
    wqk = nc.dram_tensor("wqk", [128, 8, 1024], f16, kind="ExternalInput").ap()
    wv = nc.dram_tensor("wv", [128, 8, 512], f16, kind="ExternalInput").ap()
    wout = nc.dram_tensor("wout", [128, 4, HIDDEN], f16, kind="ExternalInput").ap()
    bqk = nc.dram_tensor("bqk", [128, 8], f32, kind="ExternalInput").ap()
    outp = nc.dram_tensor("outp", [SEQ, HIDDEN], f16, kind="ExternalOutput").ap()

    with tile.TileContext(nc) as tc:
        with (
            tc.tile_pool(name="persist", bufs=1) as pers,
            tc.tile_pool(name="pp", bufs=10) as pppool,
            tc.tile_pool(name="dvetmp", bufs=2) as dvetmp,
            tc.tile_pool(name="small", bufs=2) as small,
            tc.tile_pool(name="obuf", bufs=3) as obuf,
            tc.tile_pool(name="ps_sc", bufs=2, space="PSUM") as ps_sc,
            tc.tile_pool(name="ps_ac", bufs=1, space="PSUM") as ps_ac,
            tc.tile_pool(name="ps_aux", bufs=2, space="PSUM") as ps_aux,
        ):
            xt16 = pers.tile([128, 8, SEQ], f16, tag="xt16", name="xt16")
            wqk16 = pers.tile([128, 8, 1024], f16, tag="wqk16", name="wqk16")
            wv16 = pers.tile([128, 8, 512], f16, tag="wv16", name="wv16")
            qk = [pers.tile([128, SEQ], f16, tag=f"qk{i}", name=f"qk{i}") for i in range(8)]
            vt = [pers.tile([128, HG, 128], f16, tag=f"vt{i}", name=f"vt{i}") for i in range(16)]
            aot = [pers.tile([128, SEQ], f16, tag=f"aot{i}", name=f"aot{i}") for i in range(4)]
            wout_sb = pers.tile([128, 4, HIDDEN], f16, tag="wo", name="wo")
            bqk_sb = pers.tile([128, 8], f32, tag="bqk", name="bqk")
            ones8 = pers.tile([128, HG], f16, tag="ones8", name="ones8")

            dummy = pers.tile([128, 512], f16, tag="dummy", name="dummy")
            nc.vector.memset(dummy[:], 0.0)
            nc.vector.memset(ones8[:], 1.0)
            for t in range(16):
                nc.vector.memset(vt[t][:, :, HD + 1 : 128], 0.0)
                nc.vector.tensor_copy(vt[t][:, :, HD], ones8[:])

            # input DMAs: coarse transfers (one per tensor / token-block --
            # each dma_start costs ~600ns of issue time on its engine, so
            # per-hc slicing serializes arrival), split across the two
            # HW-DGE rings (x/wv via scalar, weights via sync).
            nc.sync.dma_start(bqk_sb[:], bqk[:])
            for h, eng in ((0, nc.sync), (4, nc.scalar), (2, nc.sync), (6, nc.scalar)):
                eng.dma_start(wqk16[:, h : h + 2, :], wqk[:, h : h + 2, :])
                eng.dma_start(xt16[:, h : h + 2, 0:512], xT[:, h : h + 2, 0:512])
            nc.scalar.dma_start(wv16[:], wv[:])
            nc.sync.dma_start(xt16[:, :, 512:1024], xT[:, :, 512:1024])
            nc.scalar.dma_start(xt16[:, :, 1024:1536], xT[:, :, 1024:1536])
            nc.sync.dma_start(xt16[:, :, 1536:2048], xT[:, :, 1536:2048])
            nc.sync.dma_start(wout_sb[:], wout[:])

            def aux_psum():
                return ps_aux.tile([128, 512], f32, tag="aux", name="aux")

            # ---- emission-step builders -----------------------------------
            def qk_cell_ops(ccx, tt):
                ops = []
                cell = {}

                def mk_mm(hc, idx, cell=cell):
                    def f():
                        if "ps" not in cell:
                            cell["ps"] = aux_psum()
                        nc.tensor.matmul(
                            cell["ps"][:],
                            wqk16[:, hc, ccx * 128 : (ccx + 1) * 128],
                            xt16[:, hc, tt * 512 : (tt + 1) * 512],
                            start=(idx == 0),
                            stop=(idx == 7),
                        )
                    return f

                # consume hc in DMA-slice arrival order (pairs alternate
                # between the two rings); accumulation order is commutative
                for idx, hc in enumerate((0, 1, 4, 5, 2, 3, 6, 7)):
                    ops.append(("mm", mk_mm(hc, idx)))

                def mk_ev(cell=cell):
                    def f():
                        nc.vector.tensor_scalar_add(
                            qk[ccx][:, tt * 512 : (tt + 1) * 512],
                            cell["ps"][:],
                            bqk_sb[:, ccx : ccx + 1],
                        )
                    return f

                ops.append(("ev", mk_ev()))
                return ops

            def v_ops(t):
                ops = []
                cell = {}

                def mk_mm(hc, cell=cell):
                    def f():
                        if "ps" not in cell:
                            cell["ps"] = aux_psum()
                        nc.tensor.matmul(
                            cell["ps"][:],
                            xt16[:, hc, t * 128 : (t + 1) * 128],
                            wv16[:, hc, :],
                            start=(hc == 0),
                            stop=(hc == 7),
                        )
                    return f

                for hc in range(8):
                    ops.append(("mm", mk_mm(hc)))

                def mk_ev(cell=cell):
                    def f():
                        nc.vector.tensor_copy(
                            vt[t][:, :, 0:HD],
                            cell["ps"][:].rearrange("p (h d) -> p h d", h=HG),
                        )
                    return f

                ops.append(("ev", mk_ev()))
                return ops

            def outproj_ops(qt):
                # qt3's output DMAs issue from the scalar engine: its queue
                # is past the last exp by then and its HW-DGE ring is idle,
                # so the final transfers don't queue behind qt0-2's outputs.
                deng = nc.scalar if qt == 3 else nc.sync
                ops = []
                for t4 in range(4):
                    tch = qt * 4 + t4
                    for nt_ in range(2):
                        cell = {}

                        def mk_mm(pair_, tch=tch, nt_=nt_, cell=cell):
                            def f():
                                if "ps" not in cell:
                                    cell["ps"] = aux_psum()
                                nc.tensor.matmul(
                                    cell["ps"][:],
                                    aot[pair_][:, tch * 128 : (tch + 1) * 128],
                                    wout_sb[:, pair_, nt_ * 512 : (nt_ + 1) * 512],
                                    start=(pair_ == 0),
                                    stop=(pair_ == 3),
                                )
                            return f

                        for pair_ in range(4):
                            ops.append(("mm", mk_mm(pair_)))

                        def mk_out(tch=tch, nt_=nt_, cell=cell):
                            def f():
                                ot = obuf.tile([128, 512], f16, tag="ot", name="ot")
                                nc.vector.tensor_copy(ot[:], cell["ps"][:])
                                deng.dma_start(
                                    outp[
                                        tch * 128 : (tch + 1) * 128,
                                        nt_ * 512 : (nt_ + 1) * 512,
                                    ],
                                    ot[:],
                                )
                            return f

                        ops.append(("ev", mk_out()))
                return ops

            # ---- dripped-work queue with forced drains + evac cooldown ----
            pending = deque()
            remaining = {}
            done = set()
            mm_done_group = {}
            cur_group = [0]

            def push(key, fns):
                remaining[key] = len(fns)
                for kind, f in fns:
                    pending.append((key, kind, f))

            def pop_one():
                key, kind, f = pending.popleft()
                f()
                remaining[key] -= 1
                if kind == "mm" and remaining[key] == 1:
                    mm_done_group[key] = cur_group[0]
                if remaining[key] == 0:
                    done.add(key)

            def need(key):
                while key not in done:
                    pop_one()

            def drip(n):
                for _ in range(n):
                    if not pending:
                        return
                    key, kind, _ = pending[0]
                    if kind == "ev" and mm_done_group.get(key) == cur_group[0]:
                        return  # cooldown: evac waits a group after its MMs
                    pop_one()

            # PE warm-up: a long dummy matmul chain with no DMA deps keeps
            # the PE HAM-warm through the input-DMA wait, so the first real
            # cells run at 2.4GHz instead of 1.2.
            wps = aux_psum()
            for i in range(22):
                nc.tensor.matmul(
                    wps[:], dummy[:, 0:128], dummy[:], start=(i == 0), stop=(i == 21)
                )

            # pair-0 prologue: K/Q token-block 0 emitted directly
            for _, f in qk_cell_ops(4, 0):
                f()
            for _, f in qk_cell_ops(0, 0):
                f()
            done.update({("qk", 4, 0), ("qk", 0, 0)})

            # deadline-ordered drip list for pair 0 (K cells ~3 groups early)
            pair0_order = (
                [("vt", 0), ("qk", 4, 1), ("vt", 1), ("vt", 2), ("qk", 4, 2)]
                + [("vt", t) for t in (3, 4)]
                + [("qk", 4, 3)]
                + [("vt", t) for t in (5, 6, 7, 8)]
                + [("qk", 0, 1)]
                + [("vt", t) for t in (9, 10, 11, 12)]
                + [("qk", 0, 2)]
                + [("vt", t) for t in (13, 14, 15)]
                + [("qk", 0, 3)]
            )

            def push_key(key):
                if key[0] == "vt":
                    push(key, v_ops(key[1]))
                else:
                    push(key, qk_cell_ops(key[1], key[2]))

            def qk_pair_keys(p):
                return [
                    ("qk", 4 + p, 0),
                    ("qk", p, 0),
                    ("qk", 4 + p, 1),
                    ("qk", 4 + p, 2),
                    ("qk", 4 + p, 3),
                    ("qk", p, 1),
                    ("qk", p, 2),
                    ("qk", p, 3),
                ]

            # ---- group emitters -------------------------------------------
            state = {}

            def emit_scores_exp(pair, qt, g):
                qtile = qk[pair]
                ktile = qk[4 + pair]
                sc = ps_sc.tile([128, 1024], f32, tag="sc", name="sc")
                nc.tensor.matmul(
                    sc[:, 0:512],
                    ktile[0:64, g * 128 : (g + 1) * 128],
                    qtile[0:64, qt * 512 : (qt + 1) * 512],
                    start=True,
                    stop=True,
                    tile_position=(0, 0),
                )
                nc.tensor.matmul(
                    sc[:, 512:1024],
                    ktile[64:128, g * 128 : (g + 1) * 128],
                    qtile[64:128, qt * 512 : (qt + 1) * 512],
                    start=True,
                    stop=True,
                    tile_position=(64, 0),
                )
                pp = pppool.tile([128, 1024], f16, tag="pp", name="pp")
                if (pair, g) in DVE_CHUNKS:
                    tmpe = dvetmp.tile([128, 1024], f32, tag="et", name="et")
                    nc.vector._custom_dve(
                        exp_p1, out=tmpe[:], in0=sc[:], s0=EH, s1=EC, imm2=EK
                    )
                    nc.vector._custom_dve(exp_p2, out=pp[:], in0=tmpe[:])
                else:
                    nc.scalar.activation(pp[:], sc[:], Exp)
                state[(pair, qt, g)] = pp

            av_count = {}

            def emit_attnv(pair, qt, g):
                i = av_count.get((pair, qt), 0)
                av_count[(pair, qt)] = i + 1
                if i == 0:
                    state["accA"] = ps_ac.tile([128, 512], f32, tag="accA", name="accA")
                    state["accB"] = ps_ac.tile([128, 512], f32, tag="accB", name="accB")
                pp = state.pop((pair, qt, g))
                nc.tensor.matmul(
                    state["accA"][:],
                    vt[g][:, 2 * pair, :],
                    pp[:, 0:512],
                    start=(i == 0),
                    stop=(i == 15),
                )
                nc.tensor.matmul(
                    state["accB"][:],
                    vt[g][:, 2 * pair + 1, :],
                    pp[:, 512:1024],
                    start=(i == 0),
                    stop=(i == 15),
                )
                return i == 15

            def emit_norm(pair, qt, fine=False):
                # denominator rows first so the gpsimd broadcasts start early;
                # numer staging copies release the PSUM accumulators fast;
                # fast 18-bit reciprocal (error ~4e-6, far under budget).
                dns = []
                for acc in (state["accA"], state["accB"]):
                    dn = small.tile([1, 512], f32, tag="dn", name="dn")
                    nc.vector.tensor_copy(dn[:], acc[64:65, :])
                    dns.append(dn)
                bcs = []
                for dn in dns:
                    bc = small.tile([64, 512], f32, tag="bc", name="bc")
                    nc.gpsimd.partition_broadcast(bc[:], dn[:])
                    bcs.append(bc)
                nums = []
                for acc in (state["accA"], state["accB"]):
                    numer = small.tile([64, 512], f32, tag="numer", name="numer")
                    nc.vector.tensor_copy(numer[:], acc[0:64, :])
                    nums.append(numer)
                rcs = []
                for bc in bcs:
                    rc = small.tile([64, 512], f32, tag="rc", name="rc")
                    nc.vector.reciprocal_approx_fast(rc[:], bc[:])
                    rcs.append(rc)
                slices = (
                    [(t * 128, 128) for t in range(4)] if fine else [(0, 512)]
                )
                for off, ln in slices:
                    for (numer, rc, row0) in (
                        (nums[0], rcs[0], 0),
                        (nums[1], rcs[1], 64),
                    ):
                        nc.vector.tensor_mul(
                            aot[pair][
                                row0 : row0 + 64,
                                qt * 512 + off : qt * 512 + off + ln,
                            ],
                            numer[:, off : off + ln],
                            rc[:, off : off + ln],
                        )

            # ---- the pipelined group loop ---------------------------------
            # Scores are emitted in adjacent double-groups (one quadrant-
            # config transition per two groups instead of two -- unhidden
            # LDWEIGHTS at each 64-row/128-row switch costs ~95ns). attn@V
            # for group g is emitted 4 groups later, which also covers the
            # DVE exp chunks' longer latency without special-casing.
            av_sched = [
                (pair, qt, g)
                for pair in range(4)
                for qt in range(4)
                for g in range(16)
            ]

            def finish_avslot(slot):
                p2, q2, g2 = av_sched[slot]
                if p2 == 0:
                    need(("vt", g2))
                if emit_attnv(p2, q2, g2):
                    emit_norm(p2, q2, fine=(p2 == 3 and q2 == 3))
                    if p2 == 3:
                        push(("op", q2), outproj_ops(q2))

            G = 0
            for pair in range(4):
                if pair == 0:
                    for key in pair0_order:
                        push_key(key)
                    for key in qk_pair_keys(1):
                        push_key(key)
                elif pair < 3:
                    for key in qk_pair_keys(pair + 1):
                        push_key(key)
                for qt in range(4):
                    for dg in range(8):
                        cur_group[0] = G
                        gleft = 64 - (qt * 16 + 2 * dg)
                        g0, g1 = 2 * dg, 2 * dg + 1
                        need(("qk", 4 + pair, g0 // 4))
                        need(("qk", pair, qt))
                        emit_scores_exp(pair, qt, g0)
                        emit_scores_exp(pair, qt, g1)
                        for s in (G - 4, G - 3):
                            if s >= 0:
                                finish_avslot(s)
                        drip(2 * -(-len(pending) // max(2, gleft)))
                        G += 2
            cur_group[0] = G
            for s in range(252, 256):
                finish_avslot(s)
            while pending:
                pop_one()
    nc.compile()
    return nc


def _get_nc():
    if "nc" not in _CACHE:
        _CACHE["nc"] = _build_nc()
    return _CACHE["nc"]


def kernel(x, W_qkv, b_qkv, W_out, b_out):
    global LAST_RESULT
    from concourse.bass_utils import run_bass_kernel_spmd

    x = np.asarray(x, dtype=np.float32)
    W_qkv = np.asarray(W_qkv, dtype=np.float32)
    b_qkv = np.asarray(b_qkv, dtype=np.float32)
    W_out = np.asarray(W_out, dtype=np.float32)
    b_out = np.asarray(b_out, dtype=np.float32)

    scale = 1.0 / np.sqrt(HD)
    # [hidden, 3, heads, hd]
    w4 = W_qkv.reshape(HIDDEN, 3, HEADS, HD)
    b4 = b_qkv.reshape(3, HEADS, HD)

    in_maps = []
    for c in range(8):
        b = c // 2
        g = c % 2
        hs = slice(g * HG, (g + 1) * HG)
        wq = (w4[:, 0, hs, :] * scale).reshape(HIDDEN, 512)
        wk = w4[:, 1, hs, :].reshape(HIDDEN, 512)
        wv_ = np.ascontiguousarray(
            w4[:, 2, hs, :].reshape(8, 128, 512).transpose(1, 0, 2)
        ).astype(np.float16)
        wqk = np.ascontiguousarray(
            np.concatenate([wq, wk], axis=1).reshape(8, 128, 1024).transpose(1, 0, 2)
        ).astype(np.float16)
        bq = (b4[0, hs, :] * scale).reshape(512)
        bk = b4[1, hs, :].reshape(512)
        bqk = np.ascontiguousarray(
            np.concatenate([bq, bk]).reshape(8, 128).T
        ).astype(np.float32)
        wout_c = np.ascontiguousarray(
            W_out[g * 512 : (g + 1) * 512, :].reshape(4, 128, HIDDEN).transpose(1, 0, 2)
        ).astype(np.float16)
        xT_b = np.ascontiguousarray(
            x[b].T.reshape(8, 128, SEQ).transpose(1, 0, 2)
        ).astype(np.float16)
        in_maps.append(
            {
                "xT": xT_b,
                "wqk": wqk,
                "wv": wv_,
                "wout": wout_c,
                "bqk": bqk,
            }
        )

    nc = _get_nc()
    res = run_bass_kernel_spmd(
        nc, in_maps, core_ids=list(range(8)), trace=_TRACE
    )
    LAST_RESULT = res

    # host reduction: sum the two head-group partials per batch; fold V-bias
    # and output bias (adding b_v to V shifts every attn output row by b_v,
    # which after the out-projection is the constant b_v @ W_out).
    bv_all = b_qkv[2 * HIDDEN : 3 * HIDDEN]
    const = (b_out + bv_all @ W_out).astype(np.float32)
    out = np.empty((BATCH, SEQ, HIDDEN), dtype=np.float32)
    for b in range(BATCH):
        out[b] = (
            res.results[2 * b]["outp"].astype(np.float32)
            + res.results[2 * b + 1]["outp"].astype(np.float32)
            + const
        )
    return out


# revision 24
# speedup vs baseline: 1.0066x; 1.0066x over previous
"""Multi-head softmax attention (b=4, s=2048, d=1024, 16 heads) on 8 trn2 cores.

Sharding: 2D over (batch, head-half). Core c handles batch c//2, heads
[8*(c%2), 8*(c%2)+8). Each core computes its QKV projections, attention for
its 8 heads, and a partial output projection (row-parallel over its 512
attn-out columns). Host sums the two partials per batch.

Device schedule: a single software-pipelined loop over (pair, qt, k-chunk)
groups. Scores for two consecutive groups are emitted back-to-back (one
64-row quadrant-config transition per two groups -- each config switch
costs ~95ns of un-prefetched LDWEIGHTS), their exps follow on the ACT
engine, and attn@V consumption trails by 4 groups so slower producers never
head-of-line-block the in-order PE queue. All projection work (V tiles,
next pair's Q^T/K^T, out-proj) is dripped between groups from a
deadline-ordered queue with forced drains before each consumer; evacuation
ops are never dripped in the same group as their matmul chain (they would
block the strict DVE FIFO while waiting). Input DMA uses coarse transfers
(dma_start costs ~600ns issue time each) split across both HW-DGE rings,
and a dummy matmul chain keeps the PE HAM-warm through the DMA wait. 16 of
the 256 exp chunks (pairs 1-2, where ACT would otherwise bind) run on the
DVE via a custom 2-instruction op computing exp(x) = (c*(x+h)^2+k)^256
(minimax quadratic + 8 squarings, rel err ~7e-4). Outputs are written f16
and summed on host.

Data path is fp16 (weights/activations/P), accumulation fp32 in PSUM,
softmax normalization in fp32. Softmax max-subtraction is skipped (scores
are O(5), exp is safe well within fp16/fp32 range).
"""

import numpy as np

HIDDEN = 1024
SEQ = 2048
BATCH = 4
HEADS = 16
HG = 8  # heads per core
HD = 64  # head dim

_CACHE = {}
_TRACE = False  # test.py sets this for profiling runs
LAST_RESULT = None

# exp(x) ~= (EC*(x+EH)^2 + EK)^256 on [-6.8, 6.8], rel err <= 7.1e-4
EC = 7.5799348451255355e-06
EH = 257.6974182128906
EK = 0.49663403630256653

# (pair, g) chunks whose exp runs on DVE instead of ACT; positioned late in
# each qt so their DVE ops never queue behind the previous qt's norm chain.
DVE_GS = (10, 13)
DVE_PAIRS = (1, 2)
DVE_CHUNKS = {(p, g) for p in DVE_PAIRS for g in DVE_GS}


def _register_dve_ops():
    from concourse import dve_ops
    from concourse.dve_spec import Spec, Src0, C0, C1, C2, sq

    if any(o.name == "EXP256_P1" for o in dve_ops.OPS):
        return

    u = Src0 + C0
    p = sq(u) * C1 + C2
    for _ in range(4):
        p = sq(p)
    spec1 = Spec(
        body=p,
        reference=lambda in0, in1, s0, s1, imm2: ((in0 + s0) ** 2 * s1 + imm2)
        ** 16,
    )
    q = Src0
    for _ in range(4):
        q = sq(q)
    spec2 = Spec(body=q, reference=lambda in0, in1, s0, s1, imm2: in0**16)

    for nm, sp, sha in [
        ("EXP256_P1", spec1, "481c0b961f8e522b"),
        ("EXP256_P2", spec2, "6d6edb7498c4a68d"),
    ]:
        op = dve_ops.DveOp(nm, sp, subdim=False, uops_sha={"v3": sha})
        dve_ops.OPS.append(op)
        dve_ops._SUB_OPCODE_FOR_NAME[nm] = (
            dve_ops._CUSTOM_DVE_ROW_BASE + len(dve_ops.OPS) - 1
        )


def _build_nc():
    from collections import deque

    import concourse.mybir as mybir
    import concourse.tile as tile
    from concourse import bacc, dve_ops

    _register_dve_ops()
    exp_p1 = next(o for o in dve_ops.OPS if o.name == "EXP256_P1")
    exp_p2 = next(o for o in dve_ops.OPS if o.name == "EXP256_P2")

    f32 = mybir.dt.float32
    f16 = mybir.dt.float16
    Exp = mybir.ActivationFunctionType.Exp

    nc = bacc.Bacc("TRN2", target_bir_lowering=False, debug=False)
    xT = nc.dram_tensor("xT", [128, 8, SEQ], f16, kind="ExternalInput").ap()
    wqk = nc.dram_tensor("wqk", [128, 8, 1024], f16, kind="ExternalInput").ap()
    wv = nc.dram_tensor("wv", [128, 8, 512], f16, kind="ExternalInput").ap()
    wout = nc.dram_tensor("wout", [128, 4, HIDDEN], f16, kind="ExternalInput").ap()
    bqk = nc.dram_tensor("bqk", [128, 8], f32, kind="ExternalInput").ap()
    outp = nc.dram_tensor("outp", [SEQ, HIDDEN], f16, kind="ExternalOutput").ap()

    with tile.TileContext(nc) as tc:
        with (
            tc.tile_pool(name="persist", bufs=1) as pers,
            tc.tile_pool(name="pp", bufs=10) as pppool,
            tc.tile_pool(name="dvetmp", bufs=2) as dvetmp,
            tc.tile_pool(name="small", bufs=2) as small,
            tc.tile_pool(name="obuf", bufs=3) as obuf,
            tc.tile_pool(name="ps_sc", bufs=2, space="PSUM") as ps_sc,
            tc.tile_pool(name="ps_ac", bufs=1, space="PSUM") as ps_ac,
            tc.tile_pool(name="ps_aux", bufs=2, space="PSUM") as ps_aux,
        ):
            xt16 = pers.tile([128, 8, SEQ], f16, tag="xt16", name="xt16")
            wqk16 = pers.tile([128, 8, 1024], f16, tag="wqk16", name="wqk16")
            wv16 = pers.tile([128, 8, 512], f16, tag="wv16", name="wv16")
            qk = [pers.tile([128, SEQ], f16, tag=f"qk{i}", name=f"qk{i}") for i in range(8)]
            vt = [pers.tile([128, HG, 128], f16, tag=f"vt{i}", name=f"vt{i}") for i in range(16)]
            aot = [pers.tile([128, SEQ], f16, tag=f"aot{i}", name=f"aot{i}") for i in range(4)]
            wout_sb = pers.tile([128, 4, HIDDEN], f16, tag="wo", name="wo")
            bqk_sb = pers.tile([128, 8], f32, tag="bqk", name="bqk")
            ones8 = pers.tile([128, HG], f16, tag="ones8", name="ones8")

            dummy = pers.tile([128, 512], f16, tag="dummy", name="dummy")
            nc.vector.memset(dummy[:], 0.0)
            nc.vector.memset(ones8[:], 1.0)
            for t in range(16):
                nc.vector.memset(vt[t][:, :, HD + 1 : 128], 0.0)
                nc.vector.tensor_copy(vt[t][:, :, HD], ones8[:])

            # input DMAs: coarse transfers (one per tensor / token-block --
            # each dma_start costs ~600ns of issue time on its engine, so
            # per-hc slicing serializes arrival), split across the two
            # HW-DGE rings (x/wv via scalar, weights via sync).
            nc.sync.dma_start(bqk_sb[:], bqk[:])
            nc.sync.dma_start(wqk16[:, 0:4, :], wqk[:, 0:4, :])
            nc.scalar.dma_start(wqk16[:, 4:8, :], wqk[:, 4:8, :])
            nc.sync.dma_start(xt16[:, 0:4, 0:512], xT[:, 0:4, 0:512])
            nc.scalar.dma_start(xt16[:, 4:8, 0:512], xT[:, 4:8, 0:512])
            nc.scalar.dma_start(wv16[:], wv[:])
            nc.sync.dma_start(xt16[:, :, 512:1024], xT[:, :, 512:1024])
            nc.scalar.dma_start(xt16[:, :, 1024:1536], xT[:, :, 1024:1536])
            nc.sync.dma_start(xt16[:, :, 1536:2048], xT[:, :, 1536:2048])
            nc.sync.dma_start(wout_sb[:], wout[:])

            def aux_psum():
                return ps_aux.tile([128, 512], f32, tag="aux", name="aux")

            # ---- emission-step builders -----------------------------------
            def qk_cell_ops(ccx, tt):
                ops = []
                cell = {}

                def mk_mm(hc, idx, cell=cell):
                    def f():
                        if "ps" not in cell:
                            cell["ps"] = aux_psum()
                        nc.tensor.matmul(
                            cell["ps"][:],
                            wqk16[:, hc, ccx * 128 : (ccx + 1) * 128],
                            xt16[:, hc, tt * 512 : (tt + 1) * 512],
                            start=(idx == 0),
                            stop=(idx == 7),
                        )
                    return f

                for idx, hc in enumerate(range(8)):
                    ops.append(("mm", mk_mm(hc, idx)))

                def mk_ev(cell=cell):
                    def f():
                        nc.vector.tensor_scalar_add(
                            qk[ccx][:, tt * 512 : (tt + 1) * 512],
                            cell["ps"][:],
                            bqk_sb[:, ccx : ccx + 1],
                        )
                    return f

                ops.append(("ev", mk_ev()))
                return ops

            def v_ops(t):
                ops = []
                cell = {}

                def mk_mm(hc, cell=cell):
                    def f():
                        if "ps" not in cell:
                            cell["ps"] = aux_psum()
                        nc.tensor.matmul(
                            cell["ps"][:],
                            xt16[:, hc, t * 128 : (t + 1) * 128],
                            wv16[:, hc, :],
                            start=(hc == 0),
                            stop=(hc == 7),
                        )
                    return f

                for hc in range(8):
                    ops.append(("mm", mk_mm(hc)))

                def mk_ev(cell=cell):
                    def f():
                        nc.vector.tensor_copy(
                            vt[t][:, :, 0:HD],
                            cell["ps"][:].rearrange("p (h d) -> p h d", h=HG),
                        )
                    return f

                ops.append(("ev", mk_ev()))
                return ops

            def outproj_ops(qt):
                # qt3's output DMAs issue from the scalar engine: its queue
                # is past the last exp by then and its HW-DGE ring is idle,
                # so the final transfers don't queue behind qt0-2's outputs.
                deng = nc.scalar if qt == 3 else nc.sync
                ops = []
                for t4 in range(4):
                    tch = qt * 4 + t4
                    for nt_ in range(2):
                        cell = {}

                        def mk_mm(pair_, tch=tch, nt_=nt_, cell=cell):
                            def f():
                                if "ps" not in cell:
                                    cell["ps"] = aux_psum()
                                nc.tensor.matmul(
                                    cell["ps"][:],
                                    aot[pair_][:, tch * 128 : (tch + 1) * 128],
                                    wout_sb[:, pair_, nt_ * 512 : (nt_ + 1) * 512],
                                    start=(pair_ == 0),
                                    stop=(pair_ == 3),
                                )
                            return f

                        for pair_ in range(4):
                            ops.append(("mm", mk_mm(pair_)))

                        def mk_out(tch=tch, nt_=nt_, cell=cell):
                            def f():
                                ot = obuf.tile([128, 512], f16, tag="ot", name="ot")
                                nc.vector.tensor_copy(ot[:], cell["ps"][:])
                                deng.dma_start(
                                    outp[
                                        tch * 128 : (tch + 1) * 128,
                                        nt_ * 512 : (nt_ + 1) * 512,
                                    ],
                                    ot[:],
                                )
                            return f

                        ops.append(("ev", mk_out()))
                return ops

            # ---- dripped-work queue with forced drains + evac cooldown ----
            pending = deque()
            remaining = {}
            done = set()
            mm_done_group = {}
            cur_group = [0]

            def push(key, fns):
                remaining[key] = len(fns)
                for kind, f in fns:
                    pending.append((key, kind, f))

            def pop_one():
                key, kind, f = pending.popleft()
                f()
                remaining[key] -= 1
                if kind == "mm" and remaining[key] == 1:
                    mm_done_group[key] = cur_group[0]
                if remaining[key] == 0:
                    done.add(key)

            def need(key):
                while key not in done:
                    pop_one()

            def drip(n):
                for _ in range(n):
                    if not pending:
                        return
                    key, kind, _ = pending[0]
                    if kind == "ev" and mm_done_group.get(key) == cur_group[0]:
                        return  # cooldown: evac waits a group after its MMs
                    pop_one()

            # PE warm-up: a long dummy matmul chain with no DMA deps keeps
            # the PE HAM-warm through the input-DMA wait, so the first real
            # cells run at 2.4GHz instead of 1.2.
            wps = aux_psum()
            for i in range(40):
                nc.tensor.matmul(
                    wps[:], dummy[:, 0:128], dummy[:], start=(i == 0), stop=(i == 39)
                )

            # pair-0 prologue: K/Q token-block 0 emitted directly
            for _, f in qk_cell_ops(4, 0):
                f()
            for _, f in qk_cell_ops(0, 0):
                f()
            done.update({("qk", 4, 0), ("qk", 0, 0)})

            # deadline-ordered drip list for pair 0 (K cells ~3 groups early)
            pair0_order = (
                [("vt", 0), ("qk", 4, 1), ("vt", 1), ("vt", 2), ("qk", 4, 2)]
                + [("vt", t) for t in (3, 4)]
                + [("qk", 4, 3)]
                + [("vt", t) for t in (5, 6, 7, 8)]
                + [("qk", 0, 1)]
                + [("vt", t) for t in (9, 10, 11, 12)]
                + [("qk", 0, 2)]
                + [("vt", t) for t in (13, 14, 15)]
                + [("qk", 0, 3)]
            )

            def push_key(key):
                if key[0] == "vt":
                    push(key, v_ops(key[1]))
                else:
                    push(key, qk_cell_ops(key[1], key[2]))

            def qk_pair_keys(p):
                return [
                    ("qk", 4 + p, 0),
                    ("qk", p, 0),
                    ("qk", 4 + p, 1),
                    ("qk", 4 + p, 2),
                    ("qk", 4 + p, 3),
                    ("qk", p, 1),
                    ("qk", p, 2),
                    ("qk", p, 3),
                ]

            # ---- group emitters -------------------------------------------
            state = {}

            def emit_scores_exp(pair, qt, g):
                qtile = qk[pair]
                ktile = qk[4 + pair]
                sc = ps_sc.tile([128, 1024], f32, tag="sc", name="sc")
                nc.tensor.matmul(
                    sc[:, 0:512],
                    ktile[0:64, g * 128 : (g + 1) * 128],
                    qtile[0:64, qt * 512 : (qt + 1) * 512],
                    start=True,
                    stop=True,
                    tile_position=(0, 0),
                )
                nc.tensor.matmul(
                    sc[:, 512:1024],
                    ktile[64:128, g * 128 : (g + 1) * 128],
                    qtile[64:128, qt * 512 : (qt + 1) * 512],
                    start=True,
                    stop=True,
                    tile_position=(64, 0),
                )
                pp = pppool.tile([128, 1024], f16, tag="pp", name="pp")
                if (pair, g) in DVE_CHUNKS:
                    tmpe = dvetmp.tile([128, 1024], f32, tag="et", name="et")
                    nc.vector._custom_dve(
                        exp_p1, out=tmpe[:], in0=sc[:], s0=EH, s1=EC, imm2=EK
                    )
                    nc.vector._custom_dve(exp_p2, out=pp[:], in0=tmpe[:])
                else:
                    nc.scalar.activation(pp[:], sc[:], Exp)
                state[(pair, qt, g)] = pp

            av_count = {}

            def emit_attnv(pair, qt, g):
                i = av_count.get((pair, qt), 0)
                av_count[(pair, qt)] = i + 1
                if i == 0:
                    state["accA"] = ps_ac.tile([128, 512], f32, tag="accA", name="accA")
                    state["accB"] = ps_ac.tile([128, 512], f32, tag="accB", name="accB")
                pp = state.pop((pair, qt, g))
                nc.tensor.matmul(
                    state["accA"][:],
                    vt[g][:, 2 * pair, :],
                    pp[:, 0:512],
                    start=(i == 0),
                    stop=(i == 15),
                )
                nc.tensor.matmul(
                    state["accB"][:],
                    vt[g][:, 2 * pair + 1, :],
                    pp[:, 512:1024],
                    start=(i == 0),
                    stop=(i == 15),
                )
                return i == 15

            def emit_norm(pair, qt, fine=False):
                # denominator rows first so the gpsimd broadcasts start early;
                # numer staging copies release the PSUM accumulators fast;
                # fast 18-bit reciprocal (error ~4e-6, far under budget).
                dns = []
                for acc in (state["accA"], state["accB"]):
                    dn = small.tile([1, 512], f32, tag="dn", name="dn")
                    nc.vector.tensor_copy(dn[:], acc[64:65, :])
                    dns.append(dn)
                bcs = []
                for dn in dns:
                    bc = small.tile([64, 512], f32, tag="bc", name="bc")
                    nc.gpsimd.partition_broadcast(bc[:], dn[:])
                    bcs.append(bc)
                nums = []
                for acc in (state["accA"], state["accB"]):
                    numer = small.tile([64, 512], f32, tag="numer", name="numer")
                    nc.vector.tensor_copy(numer[:], acc[0:64, :])
                    nums.append(numer)
                rcs = []
                for bc in bcs:
                    rc = small.tile([64, 512], f32, tag="rc", name="rc")
                    nc.vector.reciprocal_approx_fast(rc[:], bc[:])
                    rcs.append(rc)
                slices = (
                    [(t * 128, 128) for t in range(4)] if fine else [(0, 512)]
                )
                for off, ln in slices:
                    for (numer, rc, row0) in (
                        (nums[0], rcs[0], 0),
                        (nums[1], rcs[1], 64),
                    ):
                        nc.vector.tensor_mul(
                            aot[pair][
                                row0 : row0 + 64,
                                qt * 512 + off : qt * 512 + off + ln,
                            ],
                            numer[:, off : off + ln],
                            rc[:, off : off + ln],
                        )

            # ---- the pipelined group loop ---------------------------------
            # Scores are emitted in adjacent double-groups (one quadrant-
            # config transition per two groups instead of two -- unhidden
            # LDWEIGHTS at each 64-row/128-row switch costs ~95ns). attn@V
            # for group g is emitted 4 groups later, which also covers the
            # DVE exp chunks' longer latency without special-casing.
            av_sched = [
                (pair, qt, g)
                for pair in range(4)
                for qt in range(4)
                for g in range(16)
            ]

            def finish_avslot(slot):
                p2, q2, g2 = av_sched[slot]
                if p2 == 0:
                    need(("vt", g2))
                if emit_attnv(p2, q2, g2):
                    emit_norm(p2, q2, fine=(p2 == 3 and q2 == 3))
                    if p2 == 3:
                        push(("op", q2), outproj_ops(q2))

            G = 0
            for pair in range(4):
                if pair == 0:
                    for key in pair0_order:
                        push_key(key)
                    for key in qk_pair_keys(1):
                        push_key(key)
                elif pair < 3:
                    for key in qk_pair_keys(pair + 1):
                        push_key(key)
                for qt in range(4):
                    for dg in range(8):
                        cur_group[0] = G
                        gleft = 64 - (qt * 16 + 2 * dg)
                        g0, g1 = 2 * dg, 2 * dg + 1
                        need(("qk", 4 + pair, g0 // 4))
                        need(("qk", pair, qt))
                        emit_scores_exp(pair, qt, g0)
                        emit_scores_exp(pair, qt, g1)
                        for s in (G - 4, G - 3):
                            if s >= 0:
                                finish_avslot(s)
                        drip(2 * -(-len(pending) // max(2, gleft)))
                        G += 2
            cur_group[0] = G
            for s in range(252, 256):
                finish_avslot(s)
            while pending:
                pop_one()
    nc.compile()
    return nc


def _get_nc():
    if "nc" not in _CACHE:
        _CACHE["nc"] = _build_nc()
    return _CACHE["nc"]


def kernel(x, W_qkv, b_qkv, W_out, b_out):
    global LAST_RESULT
    from concourse.bass_utils import run_bass_kernel_spmd

    x = np.asarray(x, dtype=np.float32)
    W_qkv = np.asarray(W_qkv, dtype=np.float32)
    b_qkv = np.asarray(b_qkv, dtype=np.float32)
    W_out = np.asarray(W_out, dtype=np.float32)
    b_out = np.asarray(b_out, dtype=np.float32)

    scale = 1.0 / np.sqrt(HD)
    # [hidden, 3, heads, hd]
    w4 = W_qkv.reshape(HIDDEN, 3, HEADS, HD)
    b4 = b_qkv.reshape(3, HEADS, HD)

    in_maps = []
    for c in range(8):
        b = c // 2
        g = c % 2
        hs = slice(g * HG, (g + 1) * HG)
        wq = (w4[:, 0, hs, :] * scale).reshape(HIDDEN, 512)
        wk = w4[:, 1, hs, :].reshape(HIDDEN, 512)
        wv_ = np.ascontiguousarray(
            w4[:, 2, hs, :].reshape(8, 128, 512).transpose(1, 0, 2)
        ).astype(np.float16)
        wqk = np.ascontiguousarray(
            np.concatenate([wq, wk], axis=1).reshape(8, 128, 1024).transpose(1, 0, 2)
        ).astype(np.float16)
        bq = (b4[0, hs, :] * scale).reshape(512)
        bk = b4[1, hs, :].reshape(512)
        bqk = np.ascontiguousarray(
            np.concatenate([bq, bk]).reshape(8, 128).T
        ).astype(np.float32)
        wout_c = np.ascontiguousarray(
            W_out[g * 512 : (g + 1) * 512, :].reshape(4, 128, HIDDEN).transpose(1, 0, 2)
        ).astype(np.float16)
        xT_b = np.ascontiguousarray(
            x[b].T.reshape(8, 128, SEQ).transpose(1, 0, 2)
        ).astype(np.float16)
        in_maps.append(
            {
                "xT": xT_b,
                "wqk": wqk,
                "wv": wv_,
                "wout": wout_c,
                "bqk": bqk,
            }
        )

    nc = _get_nc()
    res = run_bass_kernel_spmd(
        nc, in_maps, core_ids=list(range(8)), trace=_TRACE
    )
    LAST_RESULT = res

    # host reduction: sum the two head-group partials per batch; fold V-bias
    # and output bias (adding b_v to V shifts every attn output row by b_v,
    # which after the out-projection is the constant b_v @ W_out).
    bv_all = b_qkv[2 * HIDDEN : 3 * HIDDEN]
    const = (b_out + bv_all @ W_out).astype(np.float32)
    out = np.empty((BATCH, SEQ, HIDDEN), dtype=np.float32)
    for b in range(BATCH):
        out[b] = (
            res.results[2 * b]["outp"].astype(np.float32)
            + res.results[2 * b + 1]["outp"].astype(np.float32)
            + const
        )
    return out


# revision 25
# speedup vs baseline: 1.0075x; 1.0009x over previous
"""Multi-head softmax attention (b=4, s=2048, d=1024, 16 heads) on 8 trn2 cores.

Sharding: 2D over (batch, head-half). Core c handles batch c//2, heads
[8*(c%2), 8*(c%2)+8). Each core computes its QKV projections, attention for
its 8 heads, and a partial output projection (row-parallel over its 512
attn-out columns). Host sums the two partials per batch.

Device schedule: a single software-pipelined loop over (pair, qt, k-chunk)
groups. Scores for two consecutive groups are emitted back-to-back (one
64-row quadrant-config transition per two groups -- each config switch
costs ~95ns of un-prefetched LDWEIGHTS), their exps follow on the ACT
engine, and attn@V consumption trails by 4 groups so slower producers never
head-of-line-block the in-order PE queue. All projection work (V tiles,
next pair's Q^T/K^T, out-proj) is dripped between groups from a
deadline-ordered queue with forced drains before each consumer; evacuation
ops are never dripped in the same group as their matmul chain (they would
block the strict DVE FIFO while waiting). Input DMA uses coarse transfers
(dma_start costs ~600ns issue time each) split across both HW-DGE rings,
and a dummy matmul chain keeps the PE HAM-warm through the DMA wait. 16 of
the 256 exp chunks (pairs 1-2, where ACT would otherwise bind) run on the
DVE via a custom 2-instruction op computing exp(x) = (c*(x+h)^2+k)^256
(minimax quadratic + 8 squarings, rel err ~7e-4). Outputs are written f16
and summed on host.

Data path is fp16 (weights/activations/P), accumulation fp32 in PSUM,
softmax normalization in fp32. Softmax max-subtraction is skipped (scores
are O(5), exp is safe well within fp16/fp32 range).
"""

import numpy as np

HIDDEN = 1024
SEQ = 2048
BATCH = 4
HEADS = 16
HG = 8  # heads per core
HD = 64  # head dim

_CACHE = {}
_TRACE = False  # test.py sets this for profiling runs
LAST_RESULT = None

# exp(x) ~= (EC*(x+EH)^2 + EK)^256 on [-6.8, 6.8], rel err <= 7.1e-4
EC = 7.5799348451255355e-06
EH = 257.6974182128906
EK = 0.49663403630256653

# (pair, g) chunks whose exp runs on DVE instead of ACT; positioned late in
# each qt so their DVE ops never queue behind the previous qt's norm chain.
DVE_GS = (10, 13)
DVE_PAIRS = (1, 2)
DVE_CHUNKS = {(p, g) for p in DVE_PAIRS for g in DVE_GS}


def _register_dve_ops():
    from concourse import dve_ops
    from concourse.dve_spec import Spec, Src0, C0, C1, C2, sq

    if any(o.name == "EXP256_P1" for o in dve_ops.OPS):
        return

    u = Src0 + C0
    p = sq(u) * C1 + C2
    for _ in range(4):
        p = sq(p)
    spec1 = Spec(
        body=p,
        reference=lambda in0, in1, s0, s1, imm2: ((in0 + s0) ** 2 * s1 + imm2)
        ** 16,
    )
    q = Src0
    for _ in range(4):
        q = sq(q)
    spec2 = Spec(body=q, reference=lambda in0, in1, s0, s1, imm2: in0**16)

    for nm, sp, sha in [
        ("EXP256_P1", spec1, "481c0b961f8e522b"),
        ("EXP256_P2", spec2, "6d6edb7498c4a68d"),
    ]:
        op = dve_ops.DveOp(nm, sp, subdim=False, uops_sha={"v3": sha})
        dve_ops.OPS.append(op)
        dve_ops._SUB_OPCODE_FOR_NAME[nm] = (
            dve_ops._CUSTOM_DVE_ROW_BASE + len(dve_ops.OPS) - 1
        )


def _build_nc():
    from collections import deque

    import concourse.mybir as mybir
    import concourse.tile as tile
    from concourse import bacc, dve_ops

    _register_dve_ops()
    exp_p1 = next(o for o in dve_ops.OPS if o.name == "EXP256_P1")
    exp_p2 = next(o for o in dve_ops.OPS if o.name == "EXP256_P2")

    f32 = mybir.dt.float32
    f16 = mybir.dt.float16
    Exp = mybir.ActivationFunctionType.Exp

    nc = bacc.Bacc("TRN2", target_bir_lowering=False, debug=False)
    xT = nc.dram_tensor("xT", [128, 8, SEQ], f16, kind="ExternalInput").ap()
    wqk = nc.dram_tensor("wqk", [128, 8, 1024], f16, kind="ExternalInput").ap()
    wv = nc.dram_tensor("wv", [128, 8, 512], f16, kind="ExternalInput").ap()
    wout = nc.dram_tensor("wout", [128, 4, HIDDEN], f16, kind="ExternalInput").ap()
    bqk = nc.dram_tensor("bqk", [128, 8], f32, kind="ExternalInput").ap()
    outp = nc.dram_tensor("outp", [SEQ, HIDDEN], f16, kind="ExternalOutput").ap()

    with tile.TileContext(nc) as tc:
        with (
            tc.tile_pool(name="persist", bufs=1) as pers,
            tc.tile_pool(name="pp", bufs=10) as pppool,
            tc.tile_pool(name="dvetmp", bufs=2) as dvetmp,
            tc.tile_pool(name="small", bufs=2) as small,
            tc.tile_pool(name="obuf", bufs=3) as obuf,
            tc.tile_pool(name="ps_sc", bufs=2, space="PSUM") as ps_sc,
            tc.tile_pool(name="ps_ac", bufs=1, space="PSUM") as ps_ac,
            tc.tile_pool(name="ps_aux", bufs=2, space="PSUM") as ps_aux,
        ):
            xt16 = pers.tile([128, 8, SEQ], f16, tag="xt16", name="xt16")
            wqk16 = pers.tile([128, 8, 1024], f16, tag="wqk16", name="wqk16")
            wv16 = pers.tile([128, 8, 512], f16, tag="wv16", name="wv16")
            qk = [pers.tile([128, SEQ], f16, tag=f"qk{i}", name=f"qk{i}") for i in range(8)]
            vt = [pers.tile([128, HG, 128], f16, tag=f"vt{i}", name=f"vt{i}") for i in range(16)]
            aot = [pers.tile([128, SEQ], f16, tag=f"aot{i}", name=f"aot{i}") for i in range(4)]
            wout_sb = pers.tile([128, 4, HIDDEN], f16, tag="wo", name="wo")
            bqk_sb = pers.tile([128, 8], f32, tag="bqk", name="bqk")
            ones8 = pers.tile([128, HG], f16, tag="ones8", name="ones8")

            dummy = pers.tile([128, 512], f16, tag="dummy", name="dummy")
            nc.vector.memset(dummy[:], 0.0)
            nc.vector.memset(ones8[:], 1.0)
            for t in range(16):
                nc.vector.memset(vt[t][:, :, HD + 1 : 128], 0.0)
                nc.vector.tensor_copy(vt[t][:, :, HD], ones8[:])

            # input DMAs: coarse transfers (one per tensor / token-block --
            # each dma_start costs ~600ns of issue time on its engine, so
            # per-hc slicing serializes arrival), split across the two
            # HW-DGE rings (x/wv via scalar, weights via sync).
            nc.sync.dma_start(bqk_sb[:], bqk[:])
            nc.scalar.dma_start(wv16[:], wv[:])
            nc.sync.dma_start(xt16[:, :, 0:512], xT[:, :, 0:512])
            nc.sync.dma_start(wqk16[:, 0:4, :], wqk[:, 0:4, :])
            nc.scalar.dma_start(wqk16[:, 4:8, :], wqk[:, 4:8, :])
            nc.sync.dma_start(xt16[:, :, 512:1024], xT[:, :, 512:1024])
            nc.scalar.dma_start(xt16[:, :, 1024:1536], xT[:, :, 1024:1536])
            nc.sync.dma_start(xt16[:, :, 1536:2048], xT[:, :, 1536:2048])
            nc.sync.dma_start(wout_sb[:], wout[:])

            def aux_psum():
                return ps_aux.tile([128, 512], f32, tag="aux", name="aux")

            # ---- emission-step builders -----------------------------------
            def qk_cell_ops(ccx, tt):
                ops = []
                cell = {}

                def mk_mm(hc, idx, cell=cell):
                    def f():
                        if "ps" not in cell:
                            cell["ps"] = aux_psum()
                        nc.tensor.matmul(
                            cell["ps"][:],
                            wqk16[:, hc, ccx * 128 : (ccx + 1) * 128],
                            xt16[:, hc, tt * 512 : (tt + 1) * 512],
                            start=(idx == 0),
                            stop=(idx == 7),
                        )
                    return f

                for idx, hc in enumerate(range(8)):
                    ops.append(("mm", mk_mm(hc, idx)))

                def mk_ev(cell=cell):
                    def f():
                        nc.vector.tensor_scalar_add(
                            qk[ccx][:, tt * 512 : (tt + 1) * 512],
                            cell["ps"][:],
                            bqk_sb[:, ccx : ccx + 1],
                        )
                    return f

                ops.append(("ev", mk_ev()))
                return ops

            def v_ops(t):
                ops = []
                cell = {}

                def mk_mm(hc, cell=cell):
                    def f():
                        if "ps" not in cell:
                            cell["ps"] = aux_psum()
                        nc.tensor.matmul(
                            cell["ps"][:],
                            xt16[:, hc, t * 128 : (t + 1) * 128],
                            wv16[:, hc, :],
                            start=(hc == 0),
                            stop=(hc == 7),
                        )
                    return f

                for hc in range(8):
                    ops.append(("mm", mk_mm(hc)))

                def mk_ev(cell=cell):
                    def f():
                        nc.vector.tensor_copy(
                            vt[t][:, :, 0:HD],
                            cell["ps"][:].rearrange("p (h d) -> p h d", h=HG),
                        )
                    return f

                ops.append(("ev", mk_ev()))
                return ops

            def outproj_ops(qt):
                # qt3's output DMAs issue from the scalar engine: its queue
                # is past the last exp by then and its HW-DGE ring is idle,
                # so the final transfers don't queue behind qt0-2's outputs.
                deng = nc.scalar if qt == 3 else nc.sync
                ops = []
                for t4 in range(4):
                    tch = qt * 4 + t4
                    for nt_ in range(2):
                        cell = {}

                        def mk_mm(pair_, tch=tch, nt_=nt_, cell=cell):
                            def f():
                                if "ps" not in cell:
                                    cell["ps"] = aux_psum()
                                nc.tensor.matmul(
                                    cell["ps"][:],
                                    aot[pair_][:, tch * 128 : (tch + 1) * 128],
                                    wout_sb[:, pair_, nt_ * 512 : (nt_ + 1) * 512],
                                    start=(pair_ == 0),
                                    stop=(pair_ == 3),
                                )
                            return f

                        for pair_ in range(4):
                            ops.append(("mm", mk_mm(pair_)))

                        def mk_out(tch=tch, nt_=nt_, cell=cell):
                            def f():
                                ot = obuf.tile([128, 512], f16, tag="ot", name="ot")
                                nc.vector.tensor_copy(ot[:], cell["ps"][:])
                                deng.dma_start(
                                    outp[
                                        tch * 128 : (tch + 1) * 128,
                                        nt_ * 512 : (nt_ + 1) * 512,
                                    ],
                                    ot[:],
                                )
                            return f

                        ops.append(("ev", mk_out()))
                return ops

            # ---- dripped-work queue with forced drains + evac cooldown ----
            pending = deque()
            remaining = {}
            done = set()
            mm_done_group = {}
            cur_group = [0]

            def push(key, fns):
                remaining[key] = len(fns)
                for kind, f in fns:
                    pending.append((key, kind, f))

            def pop_one():
                key, kind, f = pending.popleft()
                f()
                remaining[key] -= 1
                if kind == "mm" and remaining[key] == 1:
                    mm_done_group[key] = cur_group[0]
                if remaining[key] == 0:
                    done.add(key)

            def need(key):
                while key not in done:
                    pop_one()

            def drip(n):
                for _ in range(n):
                    if not pending:
                        return
                    key, kind, _ = pending[0]
                    if kind == "ev" and mm_done_group.get(key) == cur_group[0]:
                        return  # cooldown: evac waits a group after its MMs
                    pop_one()

            # PE warm-up: a long dummy matmul chain with no DMA deps keeps
            # the PE HAM-warm through the input-DMA wait, so the first real
            # cells run at 2.4GHz instead of 1.2.
            wps = aux_psum()
            for i in range(16):
                nc.tensor.matmul(
                    wps[:], dummy[:, 0:128], dummy[:], start=(i == 0), stop=(i == 15)
                )

            # head: V tiles 0-3 (gated only on wv + x tokens 0:512, which
            # land ~5us before wqk) do real work where a dummy warmup would
            # otherwise idle, and shrink pair0's drip load.
            for t in range(4):
                for _, f in v_ops(t):
                    f()
                done.add(("vt", t))

            # pair-0 prologue: K/Q token-block 0 emitted directly
            for _, f in qk_cell_ops(4, 0):
                f()
            for _, f in qk_cell_ops(0, 0):
                f()
            done.update({("qk", 4, 0), ("qk", 0, 0)})

            # deadline-ordered drip list for pair 0 (K cells ~3 groups early)
            pair0_order = (
                [("qk", 4, 1), ("vt", 4), ("qk", 4, 2), ("vt", 5), ("vt", 6)]
                + [("qk", 4, 3)]
                + [("vt", t) for t in (7, 8)]
                + [("qk", 0, 1)]
                + [("vt", t) for t in (9, 10, 11, 12)]
                + [("qk", 0, 2)]
                + [("vt", t) for t in (13, 14, 15)]
                + [("qk", 0, 3)]
            )

            def push_key(key):
                if key[0] == "vt":
                    push(key, v_ops(key[1]))
                else:
                    push(key, qk_cell_ops(key[1], key[2]))

            def qk_pair_keys(p):
                return [
                    ("qk", 4 + p, 0),
                    ("qk", p, 0),
                    ("qk", 4 + p, 1),
                    ("qk", 4 + p, 2),
                    ("qk", 4 + p, 3),
                    ("qk", p, 1),
                    ("qk", p, 2),
                    ("qk", p, 3),
                ]

            # ---- group emitters -------------------------------------------
            state = {}

            def emit_scores_exp(pair, qt, g):
                qtile = qk[pair]
                ktile = qk[4 + pair]
                sc = ps_sc.tile([128, 1024], f32, tag="sc", name="sc")
                nc.tensor.matmul(
                    sc[:, 0:512],
                    ktile[0:64, g * 128 : (g + 1) * 128],
                    qtile[0:64, qt * 512 : (qt + 1) * 512],
                    start=True,
                    stop=True,
                    tile_position=(0, 0),
                )
                nc.tensor.matmul(
                    sc[:, 512:1024],
                    ktile[64:128, g * 128 : (g + 1) * 128],
                    qtile[64:128, qt * 512 : (qt + 1) * 512],
                    start=True,
                    stop=True,
                    tile_position=(64, 0),
                )
                pp = pppool.tile([128, 1024], f16, tag="pp", name="pp")
                if (pair, g) in DVE_CHUNKS:
                    tmpe = dvetmp.tile([128, 1024], f32, tag="et", name="et")
                    nc.vector._custom_dve(
                        exp_p1, out=tmpe[:], in0=sc[:], s0=EH, s1=EC, imm2=EK
                    )
                    nc.vector._custom_dve(exp_p2, out=pp[:], in0=tmpe[:])
                else:
                    nc.scalar.activation(pp[:], sc[:], Exp)
                state[(pair, qt, g)] = pp

            av_count = {}

            def emit_attnv(pair, qt, g):
                i = av_count.get((pair, qt), 0)
                av_count[(pair, qt)] = i + 1
                if i == 0:
                    state["accA"] = ps_ac.tile([128, 512], f32, tag="accA", name="accA")
                    state["accB"] = ps_ac.tile([128, 512], f32, tag="accB", name="accB")
                pp = state.pop((pair, qt, g))
                nc.tensor.matmul(
                    state["accA"][:],
                    vt[g][:, 2 * pair, :],
                    pp[:, 0:512],
                    start=(i == 0),
                    stop=(i == 15),
                )
                nc.tensor.matmul(
                    state["accB"][:],
                    vt[g][:, 2 * pair + 1, :],
                    pp[:, 512:1024],
                    start=(i == 0),
                    stop=(i == 15),
                )
                return i == 15

            def emit_norm(pair, qt, fine=False):
                # denominator rows first so the gpsimd broadcasts start early;
                # numer staging copies release the PSUM accumulators fast;
                # fast 18-bit reciprocal (error ~4e-6, far under budget).
                dns = []
                for acc in (state["accA"], state["accB"]):
                    dn = small.tile([1, 512], f32, tag="dn", name="dn")
                    nc.vector.tensor_copy(dn[:], acc[64:65, :])
                    dns.append(dn)
                bcs = []
                for dn in dns:
                    bc = small.tile([64, 512], f32, tag="bc", name="bc")
                    nc.gpsimd.partition_broadcast(bc[:], dn[:])
                    bcs.append(bc)
                nums = []
                for acc in (state["accA"], state["accB"]):
                    numer = small.tile([64, 512], f32, tag="numer", name="numer")
                    nc.vector.tensor_copy(numer[:], acc[0:64, :])
                    nums.append(numer)
                rcs = []
                for bc in bcs:
                    rc = small.tile([64, 512], f32, tag="rc", name="rc")
                    nc.vector.reciprocal_approx_fast(rc[:], bc[:])
                    rcs.append(rc)
                slices = (
                    [(t * 128, 128) for t in range(4)] if fine else [(0, 512)]
                )
                for off, ln in slices:
                    for (numer, rc, row0) in (
                        (nums[0], rcs[0], 0),
                        (nums[1], rcs[1], 64),
                    ):
                        nc.vector.tensor_mul(
                            aot[pair][
                                row0 : row0 + 64,
                                qt * 512 + off : qt * 512 + off + ln,
                            ],
                            numer[:, off : off + ln],
                            rc[:, off : off + ln],
                        )

            # ---- the pipelined group loop ---------------------------------
            # Scores are emitted in adjacent double-groups (one quadrant-
            # config transition per two groups instead of two -- unhidden
            # LDWEIGHTS at each 64-row/128-row switch costs ~95ns). attn@V
            # for group g is emitted 4 groups later, which also covers the
            # DVE exp chunks' longer latency without special-casing.
            av_sched = [
                (pair, qt, g)
                for pair in range(4)
                for qt in range(4)
                for g in range(16)
            ]

            def finish_avslot(slot):
                p2, q2, g2 = av_sched[slot]
                if p2 == 0:
                    need(("vt", g2))
                if emit_attnv(p2, q2, g2):
                    emit_norm(p2, q2, fine=(p2 == 3 and q2 == 3))
                    if p2 == 3:
                        push(("op", q2), outproj_ops(q2))

            G = 0
            for pair in range(4):
                if pair == 0:
                    for key in pair0_order:
                        push_key(key)
                    for key in qk_pair_keys(1):
                        push_key(key)
                elif pair < 3:
                    for key in qk_pair_keys(pair + 1):
                        push_key(key)
                for qt in range(4):
                    for dg in range(8):
                        cur_group[0] = G
                        gleft = 64 - (qt * 16 + 2 * dg)
                        g0, g1 = 2 * dg, 2 * dg + 1
                        need(("qk", 4 + pair, g0 // 4))
                        need(("qk", pair, qt))
                        emit_scores_exp(pair, qt, g0)
                        emit_scores_exp(pair, qt, g1)
                        for s in (G - 4, G - 3):
                            if s >= 0:
                                finish_avslot(s)
                        drip(2 * -(-len(pending) // max(2, gleft)))
                        G += 2
            cur_group[0] = G
            for s in range(252, 256):
                finish_avslot(s)
            while pending:
                pop_one()
    nc.compile()
    return nc


def _get_nc():
    if "nc" not in _CACHE:
        _CACHE["nc"] = _build_nc()
    return _CACHE["nc"]


def kernel(x, W_qkv, b_qkv, W_out, b_out):
    global LAST_RESULT
    from concourse.bass_utils import run_bass_kernel_spmd

    x = np.asarray(x, dtype=np.float32)
    W_qkv = np.asarray(W_qkv, dtype=np.float32)
    b_qkv = np.asarray(b_qkv, dtype=np.float32)
    W_out = np.asarray(W_out, dtype=np.float32)
    b_out = np.asarray(b_out, dtype=np.float32)

    scale = 1.0 / np.sqrt(HD)
    # [hidden, 3, heads, hd]
    w4 = W_qkv.reshape(HIDDEN, 3, HEADS, HD)
    b4 = b_qkv.reshape(3, HEADS, HD)

    in_maps = []
    for c in range(8):
        b = c // 2
        g = c % 2
        hs = slice(g * HG, (g + 1) * HG)
        wq = (w4[:, 0, hs, :] * scale).reshape(HIDDEN, 512)
        wk = w4[:, 1, hs, :].reshape(HIDDEN, 512)
        wv_ = np.ascontiguousarray(
            w4[:, 2, hs, :].reshape(8, 128, 512).transpose(1, 0, 2)
        ).astype(np.float16)
        wqk = np.ascontiguousarray(
            np.concatenate([wq, wk], axis=1).reshape(8, 128, 1024).transpose(1, 0, 2)
        ).astype(np.float16)
        bq = (b4[0, hs, :] * scale).reshape(512)
        bk = b4[1, hs, :].reshape(512)
        bqk = np.ascontiguousarray(
            np.concatenate([bq, bk]).reshape(8, 128).T
        ).astype(np.float32)
        wout_c = np.ascontiguousarray(
            W_out[g * 512 : (g + 1) * 512, :].reshape(4, 128, HIDDEN).transpose(1, 0, 2)
        ).astype(np.float16)
        xT_b = np.ascontiguousarray(
            x[b].T.reshape(8, 128, SEQ).transpose(1, 0, 2)
        ).astype(np.float16)
        in_maps.append(
            {
                "xT": xT_b,
                "wqk": wqk,
                "wv": wv_,
                "wout": wout_c,
                "bqk": bqk,
            }
        )

    nc = _get_nc()
    res = run_bass_kernel_spmd(
        nc, in_maps, core_ids=list(range(8)), trace=_TRACE
    )
    LAST_RESULT = res

    # host reduction: sum the two head-group partials per batch; fold V-bias
    # and output bias (adding b_v to V shifts every attn output row by b_v,
    # which after the out-projection is the constant b_v @ W_out).
    bv_all = b_qkv[2 * HIDDEN : 3 * HIDDEN]
    const = (b_out + bv_all @ W_out).astype(np.float32)
    out = np.empty((BATCH, SEQ, HIDDEN), dtype=np.float32)
    for b in range(BATCH):
        out[b] = (
            res.results[2 * b]["outp"].astype(np.float32)
            + res.results[2 * b + 1]["outp"].astype(np.float32)
            + const
        )
    return out
